# revision 1
# baseline (speedup 1.0000x reference)
"""LoRA linear (dropout -> x @ A.T @ B.T * scaling) on 8 TRN2 NeuronCores.

Data-parallel over tokens: each core handles T/8 = 2048 tokens; lora_A/lora_B
are replicated. All math in fp32.

Per-core pipeline, per 128-token tile:
  DMA x,u -> DVE mask=(u>=p), xd=x*mask -> PE-transpose xd (32x [128,128])
  -> ACT copy PSUM->SBUF -> matmul1 hT[64,128] (accum over 32 K-chunks)
  -> matmul2 out[128,512]x8 -> ACT copy -> DMA out.
The dropout 1/(1-p) and the LoRA alpha/r scaling are folded into lora_B on
the host, so no extra scaling pass is needed on-chip.
"""

import sys

sys.path.insert(0, "/opt/trn_rl_repo")

import numpy as np

import concourse.bacc as bacc
import concourse.bass as bass
import concourse.tile as tile
from concourse import masks, mybir
from concourse.bass_utils import run_bass_kernel_spmd

N_CORES = 8
T, IN, OUT, R = 16384, 4096, 4096, 64
TS = T // N_CORES  # tokens per core
P_DROP = 0.1
SCALE = (128.0 / 64.0) / (1.0 - P_DROP)  # alpha/r * 1/(1-p), folded into B

F32 = mybir.dt.float32
KC = IN // 128  # contraction chunks (32)
NOC = OUT // 512  # output column chunks (8)


def _emit(tc, x, u, a, b, o, ts):
    """Emit the per-core program. ts = tokens this core (multiple of 128)."""
    nc = tc.nc
    ntt = ts // 128
    from contextlib import ExitStack

    with ExitStack() as ctx:
        const = ctx.enter_context(tc.tile_pool(name="const", bufs=1))
        setup = ctx.enter_context(tc.tile_pool(name="setup", bufs=1))
        xpool = ctx.enter_context(tc.tile_pool(name="xp", bufs=2))
        upool = ctx.enter_context(tc.tile_pool(name="up", bufs=2))
        xtpool = ctx.enter_context(tc.tile_pool(name="xtp", bufs=2))
        hpool = ctx.enter_context(tc.tile_pool(name="hp", bufs=2))
        opool = ctx.enter_context(tc.tile_pool(name="op", bufs=2))
        pst = ctx.enter_context(tc.tile_pool(name="pst", bufs=3, space="PSUM"))
        psh = ctx.enter_context(tc.tile_pool(name="psh", bufs=2, space="PSUM"))
        pso = ctx.enter_context(tc.tile_pool(name="pso", bufs=2, space="PSUM"))

        ident = const.tile([128, 128], F32)
        masks.make_identity(nc, ident[:])

        # aT[:, kc*64:(kc+1)*64] = A[:, kc*128:(kc+1)*128].T   ([128 i, 64 r])
        a_nat = setup.tile([R, IN], F32, tag="setup")
        nc.sync.dma_start(a_nat[:], a[:, :])
        aT = const.tile([128, KC * R], F32)
        for kc in range(KC):
            tp = pst.tile([128, 512], F32, tag="tp")
            nc.tensor.transpose(
                tp[:, :R], a_nat[:, kc * 128 : (kc + 1) * 128], ident[:R, :R]
            )
            nc.scalar.copy(aT[:, kc * R : (kc + 1) * R], tp[:, :R])

        # bT[:, oc*128:(oc+1)*128] = B[oc*128:(oc+1)*128, :].T  ([64 r, 128 o])
        nbc = OUT // 128
        b_sb = setup.tile([128, nbc * R], F32, tag="setup")
        nc.sync.dma_start(
            b_sb[:].rearrange("p (c r) -> p c r", c=nbc),
            b.rearrange("(c p) r -> p c r", p=128),
        )
        bT = const.tile([R, OUT], F32)
        for oc in range(nbc):
            tp = pst.tile([128, 512], F32, tag="tp")
            nc.tensor.transpose(
                tp[:R, :128], b_sb[:, oc * R : (oc + 1) * R], ident[:, :]
            )
            nc.scalar.copy(bT[:, oc * 128 : (oc + 1) * 128], tp[:R, :128])

        for ti in range(ntt):
            rows = slice(ti * 128, (ti + 1) * 128)
            xt = xpool.tile([128, IN], F32)
            nc.sync.dma_start(xt[:], x[rows, :])
            ut = upool.tile([128, IN], F32)
            nc.sync.dma_start(ut[:], u[rows, :])

            # dropout: ut <- (ut >= p), xt <- xt * ut
            nc.vector.tensor_scalar(
                ut[:], ut[:], P_DROP, None, mybir.AluOpType.is_ge
            )
            nc.vector.tensor_tensor(xt[:], xt[:], ut[:], mybir.AluOpType.mult)

            # xdT[:, kc*128:(kc+1)*128] = xd[:, kc*128:(kc+1)*128].T
            xdT = xtpool.tile([128, IN], F32)
            for g in range(IN // 512):
                tp = pst.tile([128, 512], F32, tag="tp")
                for j in range(4):
                    kc = g * 4 + j
                    nc.tensor.transpose(
                        tp[:, j * 128 : (j + 1) * 128],
                        xt[:, kc * 128 : (kc + 1) * 128],
                        ident[:],
                    )
                nc.scalar.copy(xdT[:, g * 512 : (g + 1) * 512], tp[:])

            # hT[64, 128] = sum_kc aT_kc.T @ xdT_kc
            ph = psh.tile([R, 128], F32)
            for kc in range(KC):
                nc.tensor.matmul(
                    ph[:],
                    aT[:, kc * R : (kc + 1) * R],
                    xdT[:, kc * 128 : (kc + 1) * 128],
                    start=(kc == 0),
                    stop=(kc == KC - 1),
                )
            hT = hpool.tile([R, 128], F32)
            nc.vector.tensor_copy(hT[:], ph[:])

            # out[128, 512*8] = hT.T @ bT
            osb = opool.tile([128, OUT], F32)
            for oc in range(NOC):
                po = pso.tile([128, 512], F32, tag="po")
                nc.tensor.matmul(
                    po[:],
                    hT[:],
                    bT[:, oc * 512 : (oc + 1) * 512],
                    start=True,
                    stop=True,
                )
                nc.scalar.copy(osb[:, oc * 512 : (oc + 1) * 512], po[:])
            nc.sync.dma_start(o[rows, :], osb[:])


def build_nc(ts=TS):
    nc = bacc.Bacc()
    x_d = nc.declare_dram_parameter("x", [ts, IN], F32, isOutput=False)
    u_d = nc.declare_dram_parameter("u", [ts, IN], F32, isOutput=False)
    a_d = nc.declare_dram_parameter("a", [R, IN], F32, isOutput=False)
    b_d = nc.declare_dram_parameter("b", [OUT, R], F32, isOutput=False)
    o_d = nc.declare_dram_parameter("o", [ts, OUT], F32, isOutput=True)
    with tile.TileContext(nc) as tc:
        _emit(tc, x_d[:], u_d[:], a_d[:], b_d[:], o_d[:], ts)
    # run_bass_via_pjrt expects a finalized module; Bacc.finalize() also runs
    # the TRN2 sync-wait legalization (move_matmul_waits_to_ldweights etc.).
    if not nc.is_finalized():
        nc.finalize()
    return nc


_NC_CACHE = None


def _get_nc():
    global _NC_CACHE
    if _NC_CACHE is None:
        _NC_CACHE = build_nc()
    return _NC_CACHE


def _in_maps(x, lora_A, lora_B, drop_u):
    bs = np.ascontiguousarray(lora_B.astype(np.float32) * np.float32(SCALE))
    a = np.ascontiguousarray(lora_A.astype(np.float32))
    return [
        {
            "x": np.ascontiguousarray(x[c * TS : (c + 1) * TS]),
            "u": np.ascontiguousarray(drop_u[c * TS : (c + 1) * TS]),
            "a": a,
            "b": bs,
        }
        for c in range(N_CORES)
    ]


def run_spmd(x, lora_A, lora_B, drop_u, **kw):
    res = run_bass_kernel_spmd(
        _get_nc(), _in_maps(x, lora_A, lora_B, drop_u), list(range(N_CORES)), **kw
    )
    out = np.concatenate([r["o"] for r in res.results], axis=0)
    return out, res


def kernel(x, lora_A, lora_B, drop_u):
    out, _ = run_spmd(x, lora_A, lora_B, drop_u)
    return out



# revision 3
# speedup vs baseline: 2.0290x; 2.0290x over previous
"""LoRA linear (dropout -> x @ A.T @ B.T * scaling) on 8 TRN2 NeuronCores.

Data-parallel over tokens: each core handles T/8 = 2048 tokens; lora_A/lora_B
are replicated.

Precision plan: all device traffic is bf16 (48MB/core instead of 96MB fp32),
and the PE runs at 1 cycle/row instead of fp32's 4. The dropout compare stays
EXACT on device: the host ships u16 = bf16(drop_u - 0.1); bf16 shares fp32's
exponent range and the fp32 subtraction is exact near 0.1 (Sterbenz), so
sign(u16) == sign(drop_u - 0.1) bit-for-bit and the device mask is
(u16 >= 0). The 1/(1-p) and alpha/r scalings are folded into lora_B on host.

Layout plan: the host packs x and u into the exact transposed SBUF tile
layout ([i-chunk on partitions, tokens free]), so no on-chip transpose is
needed and every DMA row is 16KB contiguous. Per 256-token block:
  DMA x16,u16 -> DVE mask=(u16>=0), xd=x*mask -> 32 accumulating matmuls
  hT[64,256] -> 16 matmuls out[128,512] -> ACT cast-copy -> DMA out (bf16).
Host upcasts the bf16 output to fp32.
"""

import sys

sys.path.insert(0, "/opt/trn_rl_repo")

import ml_dtypes
import numpy as np

import concourse.bacc as bacc
import concourse.tile as tile
from concourse import mybir
from concourse.bass_utils import run_bass_kernel_spmd

N_CORES = 8
T, IN, OUT, R = 16384, 4096, 4096, 64
TS = T // N_CORES  # tokens per core (2048)
P_DROP = 0.1
SCALE = (128.0 / 64.0) / (1.0 - P_DROP)  # alpha/r * 1/(1-p), folded into B

F32 = mybir.dt.float32
BF16 = mybir.dt.bfloat16
NPBF16 = np.dtype(ml_dtypes.bfloat16)

TB = 256  # tokens per block
NB = TS // TB  # blocks per core (8)
KC = IN // 128  # contraction chunks (32)


def _emit(tc, x, u, a, b, o):
    """Per-core program. x/u are [NB*128, KC*TB] packed transposed blocks
    (row = blk*128 + p, col = kc*TB + t), a is [128, KC*64] packed A chunks,
    b is [64, OUT] = scaled B transposed, o is [TS, OUT] natural layout."""
    nc = tc.nc
    from contextlib import ExitStack

    with ExitStack() as ctx:
        const = ctx.enter_context(tc.tile_pool(name="const", bufs=1))
        xpool = ctx.enter_context(tc.tile_pool(name="xp", bufs=2))
        upool = ctx.enter_context(tc.tile_pool(name="up", bufs=2))
        mpool = ctx.enter_context(tc.tile_pool(name="mp", bufs=2))
        hpool = ctx.enter_context(tc.tile_pool(name="hp", bufs=2))
        opool = ctx.enter_context(tc.tile_pool(name="op", bufs=2))
        psh = ctx.enter_context(tc.tile_pool(name="psh", bufs=2, space="PSUM"))
        pso = ctx.enter_context(tc.tile_pool(name="pso", bufs=2, space="PSUM"))

        a_sb = const.tile([128, KC * R], BF16)
        nc.sync.dma_start(a_sb[:], a[:, :])
        b_sb = const.tile([R, OUT], BF16)
        nc.sync.dma_start(b_sb[:], b[:, :])

        for blk in range(NB):
            rows = slice(blk * 128, (blk + 1) * 128)
            xt = xpool.tile([128, KC * TB], BF16)
            nc.sync.dma_start(xt[:], x[rows, :])
            ut = upool.tile([128, KC * TB], BF16)
            nc.sync.dma_start(ut[:], u[rows, :])

            # dropout mask: mt = (u16 >= 0), exact; xt *= mt
            mt = mpool.tile([128, KC * TB], BF16)
            nc.vector.tensor_scalar(
                mt[:], ut[:], 0.0, None, mybir.AluOpType.is_ge
            )
            nc.vector.tensor_tensor(xt[:], xt[:], mt[:], mybir.AluOpType.mult)

            # hT[64, TB] = sum_kc a_kc.T @ xdT_kc  (contraction over i)
            ph = psh.tile([R, TB], F32)
            for kc in range(KC):
                nc.tensor.matmul(
                    ph[:],
                    a_sb[:, kc * R : (kc + 1) * R],
                    xt[:, kc * TB : (kc + 1) * TB],
                    start=(kc == 0),
                    stop=(kc == KC - 1),
                )
            hT = hpool.tile([R, TB], BF16)
            nc.scalar.copy(hT[:], ph[:])

            # out[TB, OUT] = hT.T @ b_sb, two 128-token halves
            osb = opool.tile([128, 2 * OUT], BF16)
            for tc2 in range(2):
                for g in range(OUT // 1024):
                    po = pso.tile([128, 1024], F32)
                    for j in range(2):
                        oc = g * 2 + j
                        nc.tensor.matmul(
                            po[:, j * 512 : (j + 1) * 512],
                            hT[:, tc2 * 128 : (tc2 + 1) * 128],
                            b_sb[:, oc * 512 : (oc + 1) * 512],
                            start=True,
                            stop=True,
                        )
                    nc.scalar.copy(
                        osb[
                            :,
                            tc2 * OUT + g * 1024 : tc2 * OUT + (g + 1) * 1024,
                        ],
                        po[:],
                    )
            nc.scalar.dma_start(
                o[blk * TB : (blk + 1) * TB, :].rearrange(
                    "(c p) o -> p c o", p=128
                ),
                osb[:].rearrange("p (c o) -> p c o", c=2),
            )


def build_nc():
    nc = bacc.Bacc()
    x_d = nc.declare_dram_parameter("x", [NB * 128, KC * TB], BF16, isOutput=False)
    u_d = nc.declare_dram_parameter("u", [NB * 128, KC * TB], BF16, isOutput=False)
    a_d = nc.declare_dram_parameter("a", [128, KC * R], BF16, isOutput=False)
    b_d = nc.declare_dram_parameter("b", [R, OUT], BF16, isOutput=False)
    o_d = nc.declare_dram_parameter("o", [TS, OUT], BF16, isOutput=True)
    with tile.TileContext(nc) as tc:
        _emit(tc, x_d[:], u_d[:], a_d[:], b_d[:], o_d[:])
    if not nc.is_finalized():
        nc.finalize()
    return nc


_NC_CACHE = None


def _get_nc():
    global _NC_CACHE
    if _NC_CACHE is None:
        _NC_CACHE = build_nc()
    return _NC_CACHE


def _pack_tokens(arr):
    """[T, IN] fp32 -> per-core [NB*128, KC*TB] bf16 in transposed block
    layout: out[c][blk*128+p, kc*TB+t] = arr[c*TS + blk*TB + t, kc*128+p]."""
    a5 = arr.reshape(N_CORES, NB, TB, KC, 128).transpose(0, 1, 4, 3, 2)
    return np.ascontiguousarray(a5.astype(NPBF16)).reshape(
        N_CORES, NB * 128, KC * TB
    )


def _in_maps(x, lora_A, lora_B, drop_u):
    xp = _pack_tokens(np.asarray(x, dtype=np.float32))
    up = _pack_tokens(np.asarray(drop_u, dtype=np.float32) - np.float32(P_DROP))
    # a[p, kc*64+r] = A[r, kc*128+p]
    ap = np.ascontiguousarray(
        np.asarray(lora_A, dtype=np.float32)
        .T.reshape(KC, 128, R)
        .transpose(1, 0, 2)
        .astype(NPBF16)
    ).reshape(128, KC * R)
    bp = np.ascontiguousarray(
        (np.asarray(lora_B, dtype=np.float32) * np.float32(SCALE))
        .T.astype(NPBF16)
    )
    return [
        {"x": xp[c], "u": up[c], "a": ap, "b": bp} for c in range(N_CORES)
    ]


def run_spmd(x, lora_A, lora_B, drop_u, **kw):
    res = run_bass_kernel_spmd(
        _get_nc(), _in_maps(x, lora_A, lora_B, drop_u), list(range(N_CORES)), **kw
    )
    out = np.concatenate(
        [np.asarray(r["o"]).astype(np.float32) for r in res.results], axis=0
    )
    return out, res


def kernel(x, lora_A, lora_B, drop_u):
    out, _ = run_spmd(x, lora_A, lora_B, drop_u)
    return out


# revision 4
# speedup vs baseline: 2.3607x; 1.1635x over previous
"""LoRA linear (dropout -> x @ A.T @ B.T * scaling) on 8 TRN2 NeuronCores.

Data-parallel over tokens: each core handles T/8 = 2048 tokens; lora_A/lora_B
are replicated.

Precision plan: all device traffic is bf16 (48MB/core instead of 96MB fp32),
and the PE runs at 1 cycle/row instead of fp32's 4. The dropout compare stays
EXACT on device: the host ships u16 = bf16(drop_u - 0.1); bf16 shares fp32's
exponent range and the fp32 subtraction is exact near 0.1 (Sterbenz), so
sign(u16) == sign(drop_u - 0.1) bit-for-bit and the device mask is
(u16 >= 0). The 1/(1-p) and alpha/r scalings are folded into lora_B on host.

Layout plan: the host packs x and u (interleaved per block) into the exact
transposed SBUF tile layout ([i-chunk on partitions, tokens free]), so no
on-chip transpose is needed and every load is one 4MB DMA with 32KB
contiguous per partition. Per 256-token block:
  DMA xu -> DVE mask=(u16>=0), xd=x*mask in 4 column chunks -> 32
  accumulating matmuls hT[64,256] (interleaved with DVE chunks) -> per
  128-token half: 8 matmuls out[128,512] + ACT cast-copy + 1MB store.
Host upcasts the bf16 output to fp32.
"""

import sys

sys.path.insert(0, "/opt/trn_rl_repo")

import ml_dtypes
import numpy as np

import concourse.bacc as bacc
import concourse.tile as tile
from concourse import mybir
from concourse.bass_utils import run_bass_kernel_spmd

N_CORES = 8
T, IN, OUT, R = 16384, 4096, 4096, 64
TS = T // N_CORES  # tokens per core (2048)
P_DROP = 0.1
SCALE = (128.0 / 64.0) / (1.0 - P_DROP)  # alpha/r * 1/(1-p), folded into B

F32 = mybir.dt.float32
BF16 = mybir.dt.bfloat16
NPBF16 = np.dtype(ml_dtypes.bfloat16)

TB = 256  # tokens per block
NB = TS // TB  # blocks per core (8)
KC = IN // 128  # contraction chunks (32)
W = KC * TB  # row width of one packed tensor (8192)
NCH = 4  # DVE column chunks per block
CH = W // NCH  # chunk width (2048)
KCH = KC // NCH  # contraction chunks per DVE chunk (8)


def _emit(tc, xu, a, b, o):
    """Per-core program. xu is [NB*128, 2*W]: row blk*128+p holds the x block
    row then the u block row, each W wide with col = kc*TB + t ->
    x[blk*TB+t, kc*128+p]. a is [128, KC*64] packed A chunks, b is [64, OUT]
    scaled B transposed, o is [TS, OUT] natural layout."""
    nc = tc.nc
    from contextlib import ExitStack

    with ExitStack() as ctx:
        const = ctx.enter_context(tc.tile_pool(name="const", bufs=1))
        xupool = ctx.enter_context(tc.tile_pool(name="xup", bufs=3))
        mpool = ctx.enter_context(tc.tile_pool(name="mp", bufs=2))
        hpool = ctx.enter_context(tc.tile_pool(name="hp", bufs=2))
        opool = ctx.enter_context(tc.tile_pool(name="op", bufs=3))
        psh = ctx.enter_context(tc.tile_pool(name="psh", bufs=2, space="PSUM"))
        pso = ctx.enter_context(tc.tile_pool(name="pso", bufs=2, space="PSUM"))

        a_sb = const.tile([128, KC * R], BF16)
        nc.gpsimd.dma_start(a_sb[:], a[:, :])
        b_sb = const.tile([R, OUT], BF16)
        nc.gpsimd.dma_start(b_sb[:], b[:, :])

        for blk in range(NB):
            rows = slice(blk * 128, (blk + 1) * 128)
            xut = xupool.tile([128, 2 * W], BF16)
            nc.sync.dma_start(xut[:], xu[rows, :])
            xt = xut[:, :W]
            ut = xut[:, W:]

            ph = psh.tile([R, TB], F32)
            for c in range(NCH):
                cs = slice(c * CH, (c + 1) * CH)
                mt = mpool.tile([128, CH], BF16)
                nc.vector.tensor_scalar(
                    mt[:], ut[:, cs], 0.0, None, mybir.AluOpType.is_ge
                )
                nc.vector.tensor_tensor(
                    xt[:, cs], xt[:, cs], mt[:], mybir.AluOpType.mult
                )
                # hT[64, TB] += a_kc.T @ xdT_kc over this chunk's kcs
                for j in range(KCH):
                    kc = c * KCH + j
                    nc.tensor.matmul(
                        ph[:],
                        a_sb[:, kc * R : (kc + 1) * R],
                        xt[:, kc * TB : (kc + 1) * TB],
                        start=(kc == 0),
                        stop=(kc == KC - 1),
                    )
            hT = hpool.tile([R, TB], BF16)
            nc.scalar.copy(hT[:], ph[:])

            # out[TB, OUT] = hT.T @ b_sb, stored per 128-token half
            for tc2 in range(2):
                osb = opool.tile([128, OUT], BF16)
                for g in range(OUT // 1024):
                    po = pso.tile([128, 1024], F32)
                    for j in range(2):
                        oc = g * 2 + j
                        nc.tensor.matmul(
                            po[:, j * 512 : (j + 1) * 512],
                            hT[:, tc2 * 128 : (tc2 + 1) * 128],
                            b_sb[:, oc * 512 : (oc + 1) * 512],
                            start=True,
                            stop=True,
                        )
                    nc.scalar.copy(
                        osb[:, g * 1024 : (g + 1) * 1024], po[:]
                    )
                nc.scalar.dma_start(
                    o[blk * TB + tc2 * 128 : blk * TB + (tc2 + 1) * 128, :],
                    osb[:],
                )


def build_nc():
    nc = bacc.Bacc()
    xu_d = nc.declare_dram_parameter("xu", [NB * 128, 2 * W], BF16, isOutput=False)
    a_d = nc.declare_dram_parameter("a", [128, KC * R], BF16, isOutput=False)
    b_d = nc.declare_dram_parameter("b", [R, OUT], BF16, isOutput=False)
    o_d = nc.declare_dram_parameter("o", [TS, OUT], BF16, isOutput=True)
    with tile.TileContext(nc) as tc:
        _emit(tc, xu_d[:], a_d[:], b_d[:], o_d[:])
    if not nc.is_finalized():
        nc.finalize()
    return nc


_NC_CACHE = None


def _get_nc():
    global _NC_CACHE
    if _NC_CACHE is None:
        _NC_CACHE = build_nc()
    return _NC_CACHE


def _pack_tokens(arr):
    """[T, IN] fp32 -> per-core [NB*128, W] bf16 in transposed block layout:
    out[c][blk*128+p, kc*TB+t] = arr[c*TS + blk*TB + t, kc*128+p]."""
    a5 = arr.reshape(N_CORES, NB, TB, KC, 128).transpose(0, 1, 4, 3, 2)
    return np.ascontiguousarray(a5.astype(NPBF16)).reshape(
        N_CORES, NB * 128, W
    )


def _in_maps(x, lora_A, lora_B, drop_u):
    xp = _pack_tokens(np.asarray(x, dtype=np.float32))
    up = _pack_tokens(np.asarray(drop_u, dtype=np.float32) - np.float32(P_DROP))
    xu = np.concatenate([xp, up], axis=2)  # [cores, NB*128, 2*W]
    # a[p, kc*64+r] = A[r, kc*128+p]
    ap = np.ascontiguousarray(
        np.asarray(lora_A, dtype=np.float32)
        .T.reshape(KC, 128, R)
        .transpose(1, 0, 2)
        .astype(NPBF16)
    ).reshape(128, KC * R)
    bp = np.ascontiguousarray(
        (np.asarray(lora_B, dtype=np.float32) * np.float32(SCALE))
        .T.astype(NPBF16)
    )
    return [{"xu": xu[c], "a": ap, "b": bp} for c in range(N_CORES)]


def run_spmd(x, lora_A, lora_B, drop_u, **kw):
    res = run_bass_kernel_spmd(
        _get_nc(), _in_maps(x, lora_A, lora_B, drop_u), list(range(N_CORES)), **kw
    )
    out = np.concatenate(
        [np.asarray(r["o"]).astype(np.float32) for r in res.results], axis=0
    )
    return out, res


def kernel(x, lora_A, lora_B, drop_u):
    out, _ = run_spmd(x, lora_A, lora_B, drop_u)
    return out
